# revision 7
# baseline (speedup 1.0000x reference)
"""ActorCriticRNN Trainium2 kernel.

Data-parallel over batch B=512 across 8 NeuronCores (64 envs/core).
Per core, three phases:
  P1: embT = relu(W_emb.T @ obs.T + b_emb) (kept in SBUF);
      xi = emb @ Wi + bi computed straight in the scan's folded layout and
      stored to DRAM as [T, 128, 768] (partition = b + 64*hh; cols r|z|n,
      each 256 wide holding that gate's H-half hh).
  P2: GRU scan, 256 sequential steps. Folded layout [128, 256]
      (partition = b + 64*hh, free = h%256). Carry transpose + done-mask
      fused into PE matmuls against diag(notdone); a second identity
      transpose emits yT = h_new.T to DRAM as [T, 128, 64k+b] for phase 3.
  P3: actor/critic heads from yT, plus dormancy partial sums.
Host: shard/pack inputs, gather, unfold, dormancy scalars in numpy.

All compute fp32. Hardcoded for T=256, B=512, OBS=256, FC=512, H=512, A=16.
"""

import sys

import numpy as np

if "/opt/trn_rl_repo" not in sys.path:
    sys.path.insert(0, "/opt/trn_rl_repo")

T, B, OBS, FC, H, A = 256, 512, 256, 512, 512, 16
NC = 8
BL = B // NC          # 64 envs per core
TCH = 8               # timesteps per row-chunk
RCH = TCH * BL        # 512 rows per chunk
NCHUNK = T // TCH     # 32 chunks

_PROGRAM = None


def _build_program():
    import concourse.bass as bass  # noqa: F401
    import concourse.tile as tile
    from concourse import bacc, mybir
    from contextlib import ExitStack

    F32 = mybir.dt.float32
    AF = mybir.ActivationFunctionType
    ALU = mybir.AluOpType
    X = mybir.AxisListType.X

    nc = bacc.Bacc("TRN2", target_bir_lowering=False, debug=False)

    # ---------------- I/O ----------------
    d_obsT = nc.dram_tensor("obsT", [OBS, T * BL], F32, kind="ExternalInput").ap()
    d_h0 = nc.dram_tensor("h0f", [128, 256], F32, kind="ExternalInput").ap()
    d_nd2 = nc.dram_tensor("nd2", [T, 128, 1], F32, kind="ExternalInput").ap()
    d_diag2 = nc.dram_tensor("diag2", [T, 128, 64], F32, kind="ExternalInput").ap()
    d_eye2 = nc.dram_tensor("eye2", [128, 64], F32, kind="ExternalInput").ap()
    d_wemb = nc.dram_tensor("wemb", [OBS, FC], F32, kind="ExternalInput").ap()
    d_wi = nc.dram_tensor("wi", [FC, 3 * H], F32, kind="ExternalInput").ap()
    d_wh = nc.dram_tensor("wh", [H, 3 * H], F32, kind="ExternalInput").ap()
    d_wact = nc.dram_tensor("wact", [H, FC], F32, kind="ExternalInput").ap()
    d_wcr = nc.dram_tensor("wcr", [H, FC], F32, kind="ExternalInput").ap()
    d_wao = nc.dram_tensor("wao", [FC, A], F32, kind="ExternalInput").ap()
    d_wco = nc.dram_tensor("wco", [FC, 1], F32, kind="ExternalInput").ap()
    d_bemb = nc.dram_tensor("bemb", [128, 4], F32, kind="ExternalInput").ap()
    d_bibc = nc.dram_tensor("bibc", [128, 768], F32, kind="ExternalInput").ap()
    d_bhn = nc.dram_tensor("bhn", [1, H], F32, kind="ExternalInput").ap()
    d_bact = nc.dram_tensor("bact", [128, 4], F32, kind="ExternalInput").ap()
    d_bcr = nc.dram_tensor("bcr", [128, 4], F32, kind="ExternalInput").ap()
    d_bao = nc.dram_tensor("bao", [16, 1], F32, kind="ExternalInput").ap()
    d_bco = nc.dram_tensor("bco", [1, 1], F32, kind="ExternalInput").ap()

    o_hid = nc.dram_tensor("hid_f", [128, 256], F32, kind="ExternalOutput").ap()
    o_am = nc.dram_tensor("am", [A, T, BL], F32, kind="ExternalOutput").ap()
    o_val = nc.dram_tensor("val", [T, BL], F32, kind="ExternalOutput").ap()
    o_demb = nc.dram_tensor("demb", [128, 256], F32, kind="ExternalOutput").ap()
    o_dact = nc.dram_tensor("dact", [128, 256], F32, kind="ExternalOutput").ap()
    o_dcr = nc.dram_tensor("dcr", [128, 256], F32, kind="ExternalOutput").ap()
    o_absy = nc.dram_tensor("absy", [128, 256], F32, kind="ExternalOutput").ap()

    # internal scratch
    d_xi = nc.dram_tensor("xi_scr", [T, 128, 768], F32).ap()
    d_yT = nc.dram_tensor("yT_scr", [T, 128, 256], F32).ap()   # [t, h%128, 64*(h//128)+b]

    with tile.TileContext(nc) as tc:
        es = ExitStack()
        wpool = es.enter_context(tc.tile_pool(name="weights", bufs=1))
        statp = es.enter_context(tc.tile_pool(name="stats", bufs=1))

        # ---- resident weights / biases ----
        wemb_sb = wpool.tile([128, 2 * FC], F32)
        for k in range(2):
            nc.sync.dma_start(wemb_sb[:, k * FC:(k + 1) * FC], d_wemb[k * 128:(k + 1) * 128, :])
        wi_sb = wpool.tile([128, 4 * 1536], F32)
        for k in range(4):
            nc.sync.dma_start(wi_sb[:, k * 1536:(k + 1) * 1536], d_wi[k * 128:(k + 1) * 128, :])
        wh_sb = wpool.tile([128, 4 * 1536], F32)
        for k in range(4):
            nc.sync.dma_start(wh_sb[:, k * 1536:(k + 1) * 1536], d_wh[k * 128:(k + 1) * 128, :])
        wact_sb = wpool.tile([128, 4 * FC], F32)
        wcr_sb = wpool.tile([128, 4 * FC], F32)
        for k in range(4):
            nc.sync.dma_start(wact_sb[:, k * FC:(k + 1) * FC], d_wact[k * 128:(k + 1) * 128, :])
            nc.sync.dma_start(wcr_sb[:, k * FC:(k + 1) * FC], d_wcr[k * 128:(k + 1) * 128, :])
        wao_sb = wpool.tile([128, 4 * A], F32)
        wco_sb = wpool.tile([128, 4], F32)
        for k in range(4):
            nc.sync.dma_start(wao_sb[:, k * A:(k + 1) * A], d_wao[k * 128:(k + 1) * 128, :])
            nc.sync.dma_start(wco_sb[:, k:k + 1], d_wco[k * 128:(k + 1) * 128, :])
        bemb_sb = wpool.tile([128, 4], F32)
        nc.sync.dma_start(bemb_sb[:], d_bemb[:])
        bibc_sb = wpool.tile([128, 768], F32)
        nc.sync.dma_start(bibc_sb[:], d_bibc[:])
        bhn_sb = wpool.tile([1, H], F32)
        nc.sync.dma_start(bhn_sb[:], d_bhn[:])
        bact_sb = wpool.tile([128, 4], F32)
        nc.sync.dma_start(bact_sb[:], d_bact[:])
        bcr_sb = wpool.tile([128, 4], F32)
        nc.sync.dma_start(bcr_sb[:], d_bcr[:])
        bao_sb = wpool.tile([16, 1], F32)
        nc.sync.dma_start(bao_sb[:], d_bao[:])
        bco_sb = wpool.tile([1, 1], F32)
        nc.sync.dma_start(bco_sb[:], d_bco[:])
        ones_sb = wpool.tile([1, 64], F32)
        nc.vector.memset(ones_sb[:], 1.0)
        eye2_sb = wpool.tile([128, 64], F32)
        nc.sync.dma_start(eye2_sb[:], d_eye2[:])

        acc_emb = statp.tile([128, 256], F32)
        acc_act = statp.tile([128, 256], F32)
        acc_cr = statp.tile([128, 256], F32)
        acc_absy = statp.tile([128, 256], F32)
        for acc in (acc_emb, acc_act, acc_cr, acc_absy):
            nc.vector.memset(acc[:], 0.0)

        # ================= PHASE 1: emb + xi =================
        with tc.tile_pool(name="p1sb", bufs=3) as p1sb, \
             tc.tile_pool(name="p1pse", bufs=2, space="PSUM") as p1pse, \
             tc.tile_pool(name="p1psx", bufs=3, space="PSUM") as p1psx, \
             tc.tile_pool(name="p1sb2", bufs=2) as p1sb2:
            for rc in range(NCHUNK):
                r0 = rc * RCH
                obsT_sb = p1sb2.tile([128, 2 * RCH], F32, tag="obsT")
                for k in range(2):
                    nc.sync.dma_start(obsT_sb[:, k * RCH:(k + 1) * RCH],
                                      d_obsT[k * 128:(k + 1) * 128, r0:r0 + RCH])
                embT_sb = p1sb2.tile([128, 4 * RCH], F32, tag="embT")
                for m in range(4):
                    ps = p1pse.tile([128, RCH], F32, tag="pse")
                    for k in range(2):
                        nc.tensor.matmul(ps[:], lhsT=wemb_sb[:, k * FC + 128 * m:k * FC + 128 * m + 128],
                                         rhs=obsT_sb[:, k * RCH:(k + 1) * RCH],
                                         start=(k == 0), stop=(k == 1))
                    nc.scalar.activation(embT_sb[:, m * RCH:(m + 1) * RCH], ps[:],
                                         AF.Relu, bias=bemb_sb[:, m:m + 1])
                # emb dormancy partial: sum over t within chunk
                for k in range(4):
                    red = p1sb.tile([128, 64], F32, tag="red")
                    nc.vector.tensor_reduce(red[:], embT_sb[:, k * RCH:(k + 1) * RCH].rearrange(
                        "p (t b) -> p b t", b=BL), axis=X, op=ALU.add)
                    nc.vector.tensor_tensor(acc_emb[:, k * 64:(k + 1) * 64],
                                            acc_emb[:, k * 64:(k + 1) * 64], red[:], ALU.add)
                # xi in scan layout: per (t', hh): psum [64, 768] = [r|z|n] cols,
                # each 256 = Wi columns [512g + 256hh : +256]
                for tp in range(TCH):
                    for hh in range(2):
                        ps = p1psx.tile([64, 768], F32, tag="psx")
                        for g in range(3):
                            for k in range(4):
                                nc.tensor.matmul(
                                    ps[:, 256 * g:256 * g + 256],
                                    lhsT=embT_sb[:, k * RCH + 64 * tp:k * RCH + 64 * tp + 64],
                                    rhs=wi_sb[:, k * 1536 + 512 * g + 256 * hh:
                                              k * 1536 + 512 * g + 256 * hh + 256],
                                    start=(k == 0 and g % 2 == 0), stop=(k == 3),
                                    skip_group_check=True)
                        xi_sb = p1sb.tile([64, 768], F32, tag="xisb")
                        nc.vector.tensor_tensor(xi_sb[:], ps[:], bibc_sb[64 * hh:64 * hh + 64, :],
                                                ALU.add)
                        nc.sync.dma_start(d_xi[rc * TCH + tp, 64 * hh:64 * hh + 64, :], xi_sb[:])

        # ================= PHASE 2: GRU scan =================
        with tc.tile_pool(name="s_in", bufs=4) as s_in, \
             tc.tile_pool(name="s_sb", bufs=2) as s_sb, \
             tc.tile_pool(name="s_ps", bufs=1, space="PSUM") as s_ps, \
             tc.tile_pool(name="s_psT", bufs=2, space="PSUM") as s_psT, \
             tc.tile_pool(name="s_psY", bufs=1, space="PSUM") as s_psY:

            h0_sb = s_sb.tile([128, 256], F32, tag="h0")
            nc.sync.dma_start(h0_sb[:], d_h0[:])
            cur_h = h0_sb

            def emit_yT(src, t_idx):
                """yT(t) = src.T via identity matmuls; store to d_yT[t]."""
                psY_A = s_psY.tile([128, 128], F32, tag="psYA")
                psY_B = s_psY.tile([128, 128], F32, tag="psYB")
                for k in range(4):
                    hh, fh = divmod(k, 2)
                    dst = psY_A if k < 2 else psY_B
                    col = (k % 2) * 64
                    nc.tensor.matmul(dst[:, col:col + 64],
                                     lhsT=src[64 * hh:64 * hh + 64, 128 * fh:128 * fh + 128],
                                     rhs=eye2_sb[64 * hh:64 * hh + 64, :],
                                     start=(col == 0), stop=(col != 0),
                                     skip_group_check=True)
                yT_sb = s_sb.tile([128, 256], F32, tag="yTsb")
                nc.scalar.copy(yT_sb[:, 0:128], psY_A[:])
                nc.scalar.copy(yT_sb[:, 128:256], psY_B[:])
                nc.sync.dma_start(d_yT[t_idx], yT_sb[:])

            for t in range(T):
                xi_sb = s_in.tile([128, 768], F32, tag="xi")
                nc.sync.dma_start(xi_sb[:], d_xi[t])
                diag_sb = s_in.tile([128, 64], F32, tag="diag")
                nc.sync.dma_start(diag_sb[:], d_diag2[t])
                nd_sb = s_in.tile([128, 1], F32, tag="nd")
                nc.sync.dma_start(nd_sb[:], d_nd2[t])

                # hmT k-tile k=2*hh+fh = (h*nd).T rows 128k..128k+127, via diag-matmul
                psT_A = s_psT.tile([128, 128], F32, tag="psTA")
                psT_B = s_psT.tile([128, 128], F32, tag="psTB")
                for k in range(4):
                    hh, fh = divmod(k, 2)
                    dst = psT_A if k < 2 else psT_B
                    col = (k % 2) * 64
                    nc.tensor.matmul(dst[:, col:col + 64],
                                     lhsT=cur_h[64 * hh:64 * hh + 64, 128 * fh:128 * fh + 128],
                                     rhs=diag_sb[64 * hh:64 * hh + 64, :],
                                     start=(col == 0), stop=(col != 0),
                                     skip_group_check=True)
                hmT_sb = s_sb.tile([128, 256], F32, tag="hmT")
                nc.scalar.copy(hmT_sb[:, 0:128], psT_A[:])
                nc.scalar.copy(hmT_sb[:, 128:256], psT_B[:])

                # yT for the PREVIOUS step's h_new (unmasked), off critical path
                if t > 0:
                    emit_yT(cur_h, t - 1)

                ps_rz = s_ps.tile([128, 512], F32, tag="psrz")
                ps_n = s_ps.tile([128, 256], F32, tag="psn")
                # r gate first (shortest path to the n-chain), then n, then z
                for k in range(4):
                    lhsT = hmT_sb[:, 64 * k:64 * k + 64]
                    wk = wh_sb[:, k * 1536:(k + 1) * 1536]
                    for hh in range(2):
                        po, fo = 64 * hh, 256 * hh
                        nc.tensor.matmul(ps_rz[po:po + 64, 0:256], lhsT=lhsT, rhs=wk[:, fo:fo + 256],
                                         start=(k == 0), stop=False, skip_group_check=True)
                for k in range(4):
                    lhsT = hmT_sb[:, 64 * k:64 * k + 64]
                    wk = wh_sb[:, k * 1536:(k + 1) * 1536]
                    for hh in range(2):
                        po, fo = 64 * hh, 256 * hh
                        nc.tensor.matmul(ps_n[po:po + 64, :], lhsT=lhsT, rhs=wk[:, 1024 + fo:1024 + fo + 256],
                                         start=(k == 0), stop=False, skip_group_check=True)
                nc.tensor.matmul(ps_n[0:64, :], lhsT=ones_sb[:, 0:64], rhs=bhn_sb[:, 0:256],
                                 start=False, stop=False, skip_group_check=True)
                nc.tensor.matmul(ps_n[64:128, :], lhsT=ones_sb[:, 0:64], rhs=bhn_sb[:, 256:512],
                                 start=False, stop=True, skip_group_check=True)
                for k in range(4):
                    lhsT = hmT_sb[:, 64 * k:64 * k + 64]
                    wk = wh_sb[:, k * 1536:(k + 1) * 1536]
                    for hh in range(2):
                        po, fo = 64 * hh, 256 * hh
                        nc.tensor.matmul(ps_rz[po:po + 64, 256:512], lhsT=lhsT, rhs=wk[:, 512 + fo:512 + fo + 256],
                                         start=False, stop=(k == 3 and hh == 1), skip_group_check=True)

                rz_act = s_sb.tile([128, 512], F32, tag="rz_act")
                nc.vector.tensor_tensor(rz_act[:], ps_rz[:], xi_sb[:, 0:512], ALU.add)
                rz_sig = s_sb.tile([128, 512], F32, tag="rz_sig")
                nc.scalar.activation(rz_sig[:], rz_act[:], AF.Sigmoid)
                t2 = s_sb.tile([128, 256], F32, tag="t2")
                nc.vector.tensor_tensor(t2[:], rz_sig[:, 0:256], ps_n[:], ALU.mult)
                n_pre = s_sb.tile([128, 256], F32, tag="n_pre")
                nc.vector.tensor_tensor(n_pre[:], t2[:], xi_sb[:, 512:768], ALU.add)
                n_t = s_sb.tile([128, 256], F32, tag="n_t")
                nc.scalar.activation(n_t[:], n_pre[:], AF.Tanh)
                d_tile = s_sb.tile([128, 256], F32, tag="d_t")
                nc.vector.scalar_tensor_tensor(d_tile[:], cur_h[:], nd_sb[:, 0:1], n_t[:],
                                               ALU.mult, ALU.subtract)
                e_tile = s_sb.tile([128, 256], F32, tag="e_t")
                nc.vector.tensor_tensor(e_tile[:], rz_sig[:, 256:512], d_tile[:], ALU.mult)
                hnew = s_sb.tile([128, 256], F32, tag="hnew")
                nc.vector.tensor_tensor(hnew[:], n_t[:], e_tile[:], ALU.add)
                cur_h = hnew

            emit_yT(cur_h, T - 1)
            nc.sync.dma_start(o_hid[:], cur_h[:])

        # ================= PHASE 3: heads =================
        with tc.tile_pool(name="p3sb", bufs=2) as p3sb, \
             tc.tile_pool(name="p3ps", bufs=4, space="PSUM") as p3ps, \
             tc.tile_pool(name="p3ps2", bufs=2, space="PSUM") as p3ps2, \
             tc.tile_pool(name="p3out", bufs=3) as p3out:
            for rc in range(NCHUNK):
                yT_sb = p3sb.tile([128, 4 * RCH], F32, tag="yT")
                for k in range(4):
                    nc.sync.dma_start(
                        yT_sb[:, k * RCH:(k + 1) * RCH].rearrange("p (t b) -> p t b", b=BL),
                        d_yT.rearrange("t p c -> p t c")[:, rc * TCH:(rc + 1) * TCH,
                                                        64 * k:64 * k + 64])
                actT_sb = p3sb.tile([128, 4 * RCH], F32, tag="actT")
                crT_sb = p3sb.tile([128, 4 * RCH], F32, tag="crT")
                for (wsb, bsb, dst) in ((wact_sb, bact_sb, actT_sb), (wcr_sb, bcr_sb, crT_sb)):
                    for m in range(4):
                        ps = p3ps.tile([128, RCH], F32, tag="ps")
                        for k in range(4):
                            nc.tensor.matmul(ps[:], lhsT=wsb[:, k * FC + 128 * m:k * FC + 128 * m + 128],
                                             rhs=yT_sb[:, k * RCH:(k + 1) * RCH],
                                             start=(k == 0), stop=(k == 3))
                        nc.scalar.activation(dst[:, m * RCH:(m + 1) * RCH], ps[:],
                                             AF.Relu, bias=bsb[:, m:m + 1])
                ps_am = p3ps2.tile([16, RCH], F32, tag="psam")
                for k in range(4):
                    nc.tensor.matmul(ps_am[:], lhsT=wao_sb[:, k * A:(k + 1) * A],
                                     rhs=actT_sb[:, k * RCH:(k + 1) * RCH],
                                     start=(k == 0), stop=(k == 3))
                am_sb = p3out.tile([16, RCH], F32, tag="am")
                nc.scalar.activation(am_sb[:], ps_am[:], AF.Identity, bias=bao_sb[:, 0:1])
                nc.sync.dma_start(
                    o_am[:, rc * TCH:(rc + 1) * TCH, :],
                    am_sb[:].rearrange("p (t b) -> p t b", b=BL))
                ps_v = p3ps2.tile([1, RCH], F32, tag="psv")
                for k in range(4):
                    nc.tensor.matmul(ps_v[:], lhsT=wco_sb[:, k:k + 1],
                                     rhs=crT_sb[:, k * RCH:(k + 1) * RCH],
                                     start=(k == 0), stop=(k == 3))
                val_sb = p3out.tile([1, RCH], F32, tag="val")
                nc.scalar.activation(val_sb[:], ps_v[:], AF.Identity, bias=bco_sb[:, 0:1])
                nc.sync.dma_start(
                    o_val[rc * TCH:(rc + 1) * TCH].unsqueeze(0),
                    val_sb[:].rearrange("p (t b) -> p t b", b=BL))
                # dormancy partials
                for (src, acc, use_abs) in ((actT_sb, acc_act, False), (crT_sb, acc_cr, False),
                                            (yT_sb, acc_absy, True)):
                    for k in range(4):
                        red = p3out.tile([128, 64], F32, tag="red")
                        nc.vector.tensor_reduce(red[:], src[:, k * RCH:(k + 1) * RCH].rearrange(
                            "p (t b) -> p b t", b=BL), axis=X, op=ALU.add,
                            apply_absolute_value=use_abs)
                        nc.vector.tensor_tensor(acc[:, k * 64:(k + 1) * 64],
                                                acc[:, k * 64:(k + 1) * 64], red[:], ALU.add)

        nc.sync.dma_start(o_demb[:], acc_emb[:])
        nc.sync.dma_start(o_dact[:], acc_act[:])
        nc.sync.dma_start(o_dcr[:], acc_cr[:])
        nc.sync.dma_start(o_absy[:], acc_absy[:])
        es.close()

    nc.compile()
    return nc


def _get_program():
    global _PROGRAM
    if _PROGRAM is None:
        _PROGRAM = _build_program()
    return _PROGRAM


def _fold(x):  # [b, 512] -> [2*b, 256]
    return np.ascontiguousarray(np.concatenate([x[:, :256], x[:, 256:]], axis=0))


def _unfold(xf):  # [128, 256] -> [64, 512]
    return np.concatenate([xf[:64], xf[64:]], axis=1)


def make_in_maps(hidden, obs, dones, W_emb, b_emb, Wi, bi, Wh, bhn,
                 W_act, b_act, W_act_out, b_act_out,
                 W_cr, b_cr, W_cr_out, b_cr_out):
    f32 = np.float32
    hidden = np.asarray(hidden, f32)
    obs = np.asarray(obs, f32)
    nd = (1.0 - np.asarray(dones, f32)).astype(f32)          # [T, B]
    bi_g = np.asarray(bi, f32).reshape(3, 2, 256)            # [gate, hh, 256]
    bibc = np.empty((128, 768), f32)
    for hh in range(2):
        row = bi_g[:, hh, :].reshape(768)                    # r|z|n for this hh
        bibc[64 * hh:64 * hh + 64] = row[None, :]
    eye2 = np.ascontiguousarray(np.concatenate([np.eye(64, dtype=f32)] * 2, axis=0))
    shared = dict(
        wemb=np.ascontiguousarray(W_emb, f32), wi=np.ascontiguousarray(Wi, f32),
        wh=np.ascontiguousarray(Wh, f32), wact=np.ascontiguousarray(W_act, f32),
        wcr=np.ascontiguousarray(W_cr, f32), wao=np.ascontiguousarray(W_act_out, f32),
        wco=np.ascontiguousarray(W_cr_out, f32),
        bemb=np.ascontiguousarray(np.asarray(b_emb, f32).reshape(4, 128).T),
        bibc=bibc,
        bhn=np.ascontiguousarray(np.asarray(bhn, f32)[None, :]),
        bact=np.ascontiguousarray(np.asarray(b_act, f32).reshape(4, 128).T),
        bcr=np.ascontiguousarray(np.asarray(b_cr, f32).reshape(4, 128).T),
        bao=np.ascontiguousarray(np.asarray(b_act_out, f32).reshape(A, 1)),
        bco=np.ascontiguousarray(np.asarray(b_cr_out, f32).reshape(1, 1)),
        eye2=eye2,
    )
    ii = np.arange(64)
    in_maps = []
    for c in range(NC):
        sl = slice(c * BL, (c + 1) * BL)
        obs_c = obs[:, sl, :]                                  # [T, 64, 256]
        obsT = np.ascontiguousarray(obs_c.transpose(2, 0, 1).reshape(OBS, T * BL))
        nd_c = nd[:, sl]                                       # [T, 64]
        diag = np.zeros((T, 64, 64), f32)
        diag[:, ii, ii] = nd_c
        diag2 = np.ascontiguousarray(np.concatenate([diag, diag], axis=1))  # [T,128,64]
        nd2 = np.ascontiguousarray(np.concatenate([nd_c, nd_c], axis=1))[..., None]  # [T,128,1]
        in_maps.append(dict(shared,
                            obsT=obsT, h0f=_fold(hidden[sl]),
                            nd2=nd2, diag2=diag2))
    return in_maps


def postprocess(results, log_std):
    f32 = np.float32
    hidden = np.concatenate([_unfold(r["hid_f"]) for r in results], axis=0)  # [B, H]
    actor_mean = np.concatenate(
        [np.ascontiguousarray(r["am"].transpose(1, 2, 0)) for r in results], axis=1)  # [T, B, A]
    value = np.concatenate([r["val"] for r in results], axis=1)              # [T, B]
    sigma = np.exp(np.asarray(log_std, f32)).astype(f32)

    def unstat(key):  # -> [B, 512] ; stat[p, 64k+b] = val(b of this core, 128k+p)
        full = np.empty((B, 512), f32)
        for c, r in enumerate(results):
            s = r[key].reshape(128, 4, 64)                    # [p, k, b]
            full[c * BL:(c + 1) * BL] = s.transpose(2, 1, 0).reshape(BL, 512)
        return full

    d_emb = unstat("demb")
    d_act = unstat("dact")
    d_cr = unstat("dcr")
    d_absy = unstat("absy")

    def dorm(d_sum, lead, layer_dim):
        d = (d_sum / f32(lead)).astype(f32)
        s = (d.sum(dtype=f32) / f32(layer_dim) + f32(1e-8)).astype(f32)
        mask = (d / s) <= f32(0.0)
        return f32(mask.sum() / layer_dim * 100)

    ad1 = dorm(d_act, T, FC)
    ed1 = dorm(d_emb, T, FC)
    dh = np.abs(hidden).sum(axis=0, dtype=f32)
    sh = (dh / f32(B))
    hd1 = f32(((sh / (sh.sum(dtype=f32) / f32(H) + f32(1e-8))) <= 0).sum() / H * 100)
    ed2 = dorm(d_absy, T, H)
    cd1 = dorm(d_cr, T, 256)
    return (hidden, actor_mean, sigma, value, ad1, ed1, hd1, ed2, cd1)


def kernel(hidden, obs, dones, W_emb, b_emb, Wi, bi, Wh, bhn,
           W_act, b_act, W_act_out, b_act_out, log_std,
           W_cr, b_cr, W_cr_out, b_cr_out):
    from concourse.bass_utils import run_bass_kernel_spmd
    nc = _get_program()
    in_maps = make_in_maps(hidden, obs, dones, W_emb, b_emb, Wi, bi, Wh, bhn,
                           W_act, b_act, W_act_out, b_act_out,
                           W_cr, b_cr, W_cr_out, b_cr_out)
    res = run_bass_kernel_spmd(nc, in_maps, core_ids=list(range(NC)))
    return postprocess(res.results, log_std)


# revision 13
# speedup vs baseline: 1.1762x; 1.1762x over previous
"""ActorCriticRNN Trainium2 kernel (v2).

Data-parallel over batch B=512 across 8 NeuronCores (64 envs/core).
Single fused device program per core:
  - embT = relu(W_emb.T @ obs.T + b_emb)  (SBUF only, f32r)
  - xi = emb @ Wi + bi computed in batch-major layout, 2 timesteps per
    matmul group (M=128), stored to DRAM as [T, 64, 1536] fp32.
  - GRU scan, 256 sequential steps, unfolded [64, 512] tiles.
    Gate matmuls in float32r (PE full rate); carry state kept as f32r
    pairs [128, 512] (two timesteps stacked) so yT transposes batch 2
    steps. done-mask + transpose fused into PE matmuls vs diag(notdone).
    tanh eliminated via tanh(x) = 2*sigmoid(2x)-1 so the scalar engine
    never reloads its activation table (everything in sigmoid_and_friends).
  - actor/critic heads consume yT straight from SBUF (no DRAM round trip),
    interleaved into the scan's idle engine slots two chunks behind.
Host: shard/pack inputs, gather, dormancy scalars in numpy.
"""

import sys

import numpy as np

if "/opt/trn_rl_repo" not in sys.path:
    sys.path.insert(0, "/opt/trn_rl_repo")

T, B, OBS, FC, H, A = 256, 512, 256, 512, 512, 16
NC = 8
BL = B // NC          # 64 envs per core
TCH = 8               # timesteps per chunk
RCH = TCH * BL        # 512 rows per chunk
NCHUNK = T // TCH     # 32 chunks
USE_F32R = True       # float32r matmuls (4x PE throughput, ~1e-3 matmul rounding)

_PROGRAM = None


def _build_program():
    import concourse.bass as bass  # noqa: F401
    import concourse.tile as tile
    from concourse import bacc, mybir
    from contextlib import ExitStack

    F32 = mybir.dt.float32
    F32R = mybir.dt.float32r if USE_F32R else mybir.dt.float32
    AF = mybir.ActivationFunctionType
    ALU = mybir.AluOpType
    X = mybir.AxisListType.X

    nc = bacc.Bacc("TRN2", target_bir_lowering=False, debug=False)

    # ---------------- I/O ----------------
    d_obsT = nc.dram_tensor("obsT", [OBS, T * BL], F32, kind="ExternalInput").ap()
    d_h0 = nc.dram_tensor("h0f", [BL, H], F32, kind="ExternalInput").ap()
    d_dgnd = nc.dram_tensor("dgnd", [T, 128, 65], F32R, kind="ExternalInput").ap()
    d_eye128 = nc.dram_tensor("eye128", [128, 128], F32R, kind="ExternalInput").ap()
    d_wemb = nc.dram_tensor("wemb", [OBS, FC], F32, kind="ExternalInput").ap()
    d_wi = nc.dram_tensor("wi", [FC, 3 * H], F32, kind="ExternalInput").ap()
    d_wh = nc.dram_tensor("wh", [H, 3 * H], F32, kind="ExternalInput").ap()
    d_wact = nc.dram_tensor("wact", [H, FC], F32, kind="ExternalInput").ap()
    d_wcr = nc.dram_tensor("wcr", [H, FC], F32, kind="ExternalInput").ap()
    d_wao = nc.dram_tensor("wao", [FC, A], F32, kind="ExternalInput").ap()
    d_wco = nc.dram_tensor("wco", [FC, 1], F32, kind="ExternalInput").ap()
    d_bemb = nc.dram_tensor("bemb", [128, 4], F32, kind="ExternalInput").ap()
    d_bibc = nc.dram_tensor("bibc", [128, 1536], F32, kind="ExternalInput").ap()
    d_bhn = nc.dram_tensor("bhn", [1, H], F32, kind="ExternalInput").ap()
    d_bact = nc.dram_tensor("bact", [128, 4], F32, kind="ExternalInput").ap()
    d_bcr = nc.dram_tensor("bcr", [128, 4], F32, kind="ExternalInput").ap()
    d_bao = nc.dram_tensor("bao", [16, 1], F32, kind="ExternalInput").ap()
    d_bco = nc.dram_tensor("bco", [1, 1], F32, kind="ExternalInput").ap()

    o_hid = nc.dram_tensor("hid", [BL, H], F32, kind="ExternalOutput").ap()
    o_am = nc.dram_tensor("am", [A, T, BL], F32, kind="ExternalOutput").ap()
    o_val = nc.dram_tensor("val", [T, BL], F32, kind="ExternalOutput").ap()
    o_demb = nc.dram_tensor("demb", [128, 256], F32, kind="ExternalOutput").ap()
    o_dact = nc.dram_tensor("dact", [128, 256], F32, kind="ExternalOutput").ap()
    o_dcr = nc.dram_tensor("dcr", [128, 256], F32, kind="ExternalOutput").ap()
    o_absy = nc.dram_tensor("absy", [128, 256], F32, kind="ExternalOutput").ap()

    d_xi = nc.dram_tensor("xi_scr", [T, BL, 3 * H], F32).ap()

    with tile.TileContext(nc) as tc:
        es = ExitStack()
        wpool = es.enter_context(tc.tile_pool(name="weights", bufs=1))
        statp = es.enter_context(tc.tile_pool(name="stats", bufs=1))

        # ---- load + round weights (scratch pool closed right after) ----
        wscr_pool = tc.tile_pool(name="wscr", bufs=2)
        wscr = wscr_pool.__enter__()

        def load_round(dram_ap, rows, cols, rtile):
            nk = rows // 128
            for k in range(nk):
                scr = wscr.tile([128, cols], F32, tag="wscratch")
                nc.sync.dma_start(scr[:], dram_ap[k * 128:(k + 1) * 128, :])
                nc.vector.tensor_copy(rtile[:, k * cols:(k + 1) * cols], scr[:])

        wemb_r = wpool.tile([128, 2 * FC], F32R)
        load_round(d_wemb, OBS, FC, wemb_r)
        wi_r = wpool.tile([128, 4 * 1536], F32R)
        load_round(d_wi, FC, 1536, wi_r)
        wh_r = wpool.tile([128, 4 * 1536], F32R)
        load_round(d_wh, H, 1536, wh_r)
        wact_r = wpool.tile([128, 4 * FC], F32R)
        load_round(d_wact, H, FC, wact_r)
        wcr_r = wpool.tile([128, 4 * FC], F32R)
        load_round(d_wcr, H, FC, wcr_r)
        wao_r = wpool.tile([128, 4 * A], F32R)
        wco_r = wpool.tile([128, 4], F32R)
        for k in range(4):
            scr = wscr.tile([128, A], F32, tag="wscr2")
            nc.sync.dma_start(scr[:, 0:A], d_wao[k * 128:(k + 1) * 128, :])
            nc.vector.tensor_copy(wao_r[:, k * A:(k + 1) * A], scr[:, 0:A])
            scr2 = wscr.tile([128, 1], F32, tag="wscr3")
            nc.sync.dma_start(scr2[:], d_wco[k * 128:(k + 1) * 128, :])
            nc.vector.tensor_copy(wco_r[:, k:k + 1], scr2[:])
        bhn_r = wpool.tile([1, H], F32R)
        scr3 = wscr.tile([1, H], F32, tag="wscr4")
        nc.sync.dma_start(scr3[:], d_bhn[:])
        nc.vector.tensor_copy(bhn_r[:], scr3[:])
        ones_r = wpool.tile([1, 64], F32R)
        scr4 = wscr.tile([1, 64], F32, tag="wscr5")
        nc.vector.memset(scr4[:], 1.0)
        nc.vector.tensor_copy(ones_r[:], scr4[:])
        eye_r = wpool.tile([128, 128], F32R)
        nc.sync.dma_start(eye_r[:], d_eye128[:])

        bemb_sb = wpool.tile([128, 4], F32)
        nc.sync.dma_start(bemb_sb[:], d_bemb[:])
        bibc_sb = wpool.tile([128, 1536], F32)
        nc.sync.dma_start(bibc_sb[:], d_bibc[:])
        bact_sb = wpool.tile([128, 4], F32)
        nc.sync.dma_start(bact_sb[:], d_bact[:])
        bcr_sb = wpool.tile([128, 4], F32)
        nc.sync.dma_start(bcr_sb[:], d_bcr[:])
        bao_sb = wpool.tile([16, 1], F32)
        nc.sync.dma_start(bao_sb[:], d_bao[:])
        bco_sb = wpool.tile([1, 1], F32)
        nc.sync.dma_start(bco_sb[:], d_bco[:])

        h0_f = wscr.tile([BL, H], F32, tag="h0f")
        nc.sync.dma_start(h0_f[:], d_h0[:])
        h0_r = wpool.tile([BL, H], F32R)
        nc.vector.tensor_copy(h0_r[:], h0_f[:])
        wscr_pool.__exit__(None, None, None)

        phsb = es.enter_context(tc.tile_pool(name="phsb", bufs=2))      # phase sbuf tiles
        phsb1 = es.enter_context(tc.tile_pool(name="phsb1", bufs=1))
        phsb3 = es.enter_context(tc.tile_pool(name="phsb3", bufs=2))
        phps = es.enter_context(tc.tile_pool(name="phps", bufs=3, space="PSUM"))
        ytp = es.enter_context(tc.tile_pool(name="ytp", bufs=2))
        s_in = es.enter_context(tc.tile_pool(name="s_in", bufs=2))
        s_sb = es.enter_context(tc.tile_pool(name="s_sb", bufs=1))
        s_sb2 = es.enter_context(tc.tile_pool(name="s_sb2", bufs=2))
        s_hp = es.enter_context(tc.tile_pool(name="s_hp", bufs=2))
        s_ps = es.enter_context(tc.tile_pool(name="s_ps", bufs=1, space="PSUM"))
        s_psT = es.enter_context(tc.tile_pool(name="s_psT", bufs=1, space="PSUM"))
        s_psY = es.enter_context(tc.tile_pool(name="s_psY", bufs=1, space="PSUM"))

        acc_emb = statp.tile([128, 256], F32)
        acc_act = statp.tile([128, 256], F32)
        acc_cr = statp.tile([128, 256], F32)
        acc_absy = statp.tile([128, 256], F32)
        for acc in (acc_emb, acc_act, acc_cr, acc_absy):
            nc.vector.memset(acc[:], 0.0)

        # ---------------- phase-1 chunk ----------------
        def emit_p1(rc):
            r0 = rc * RCH
            obsT_r = phsb.tile([128, 2 * RCH], F32R, tag="obsTr")
            for k in range(2):
                obsT_f = phsb.tile([128, RCH], F32, tag="obsTf")
                nc.sync.dma_start(obsT_f[:],
                                  d_obsT[k * 128:(k + 1) * 128, r0:r0 + RCH])
                nc.vector.tensor_copy(obsT_r[:, k * RCH:(k + 1) * RCH], obsT_f[:])
            embT_r = phsb.tile([128, 4 * RCH], F32R, tag="embT")
            for m in range(4):
                ps = phps.tile([128, RCH], F32, tag="ph")
                for k in range(2):
                    nc.tensor.matmul(ps[:], lhsT=wemb_r[:, k * FC + 128 * m:k * FC + 128 * m + 128],
                                     rhs=obsT_r[:, k * RCH:(k + 1) * RCH],
                                     start=(k == 0), stop=(k == 1))
                nc.scalar.activation(embT_r[:, m * RCH:(m + 1) * RCH], ps[:],
                                     AF.Relu, bias=bemb_sb[:, m:m + 1])
            for k in range(4):
                red = phsb3.tile([128, 64], F32, tag="red")
                nc.vector.tensor_reduce(red[:], embT_r[:, k * RCH:(k + 1) * RCH].bitcast(F32).rearrange(
                    "p (t b) -> p b t", b=BL), axis=X, op=ALU.add)
                nc.vector.tensor_tensor(acc_emb[:, k * 64:(k + 1) * 64],
                                        acc_emb[:, k * 64:(k + 1) * 64], red[:], ALU.add)
            # xi: two timesteps per matmul group (M = 128)
            for tp2 in range(TCH // 2):
                xi_stage = phsb3.tile([128, 1536], F32, tag="xistage")
                for g in range(3):
                    ps = phps.tile([128, RCH], F32, tag="ph")
                    for k in range(4):
                        nc.tensor.matmul(
                            ps[:], lhsT=embT_r[:, k * RCH + 128 * tp2:k * RCH + 128 * tp2 + 128],
                            rhs=wi_r[:, k * 1536 + 512 * g:k * 1536 + 512 * g + 512],
                            start=(k == 0), stop=(k == 3))
                    nc.vector.tensor_tensor(xi_stage[:, 512 * g:512 * (g + 1)], ps[:],
                                            bibc_sb[:, 512 * g:512 * (g + 1)], ALU.add)
                t0 = rc * TCH + 2 * tp2
                nc.sync.dma_start(
                    d_xi[t0:t0 + 2].rearrange("t b f -> (t b) f"), xi_stage[:])

        # ---------------- phase-3 chunk ----------------
        def emit_p3(rc, yt_chunk):
            actT_r = phsb1.tile([128, 4 * RCH], F32R, tag="actT")
            crT_r = phsb1.tile([128, 4 * RCH], F32R, tag="crT")
            for (wsb, bsb, dst) in ((wact_r, bact_sb, actT_r), (wcr_r, bcr_sb, crT_r)):
                for m in range(4):
                    ps = phps.tile([128, RCH], F32, tag="ph")
                    for k in range(4):
                        nc.tensor.matmul(ps[:], lhsT=wsb[:, k * FC + 128 * m:k * FC + 128 * m + 128],
                                         rhs=yt_chunk[:, k * RCH:(k + 1) * RCH],
                                         start=(k == 0), stop=(k == 3))
                    nc.scalar.activation(dst[:, m * RCH:(m + 1) * RCH], ps[:],
                                         AF.Relu, bias=bsb[:, m:m + 1])
            ps_am = phps.tile([16, RCH], F32, tag="ph")
            for k in range(4):
                nc.tensor.matmul(ps_am[:], lhsT=wao_r[:, k * A:(k + 1) * A],
                                 rhs=actT_r[:, k * RCH:(k + 1) * RCH],
                                 start=(k == 0), stop=(k == 3))
            am_sb = phsb3.tile([16, RCH], F32, tag="am")
            nc.scalar.activation(am_sb[:], ps_am[:], AF.Identity, bias=bao_sb[:, 0:1])
            nc.sync.dma_start(o_am[:, rc * TCH:(rc + 1) * TCH, :],
                              am_sb[:].rearrange("p (t b) -> p t b", b=BL))
            ps_v = phps.tile([1, RCH], F32, tag="ph")
            for k in range(4):
                nc.tensor.matmul(ps_v[:], lhsT=wco_r[:, k:k + 1],
                                 rhs=crT_r[:, k * RCH:(k + 1) * RCH],
                                 start=(k == 0), stop=(k == 3))
            val_sb = phsb3.tile([1, RCH], F32, tag="val")
            nc.scalar.activation(val_sb[:], ps_v[:], AF.Identity, bias=bco_sb[:, 0:1])
            nc.sync.dma_start(o_val[rc * TCH:(rc + 1) * TCH].unsqueeze(0),
                              val_sb[:].rearrange("p (t b) -> p t b", b=BL))
            for (src, acc, use_abs) in ((actT_r, acc_act, False), (crT_r, acc_cr, False),
                                        (yt_chunk, acc_absy, True)):
                for k in range(4):
                    red = phsb3.tile([128, 64], F32, tag="red")
                    nc.vector.tensor_reduce(red[:], src[:, k * RCH:(k + 1) * RCH].bitcast(F32).rearrange(
                        "p (t b) -> p b t", b=BL), axis=X, op=ALU.add,
                        apply_absolute_value=use_abs)
                    nc.vector.tensor_tensor(acc[:, k * 64:(k + 1) * 64],
                                            acc[:, k * 64:(k + 1) * 64], red[:], ALU.add)

        # ---------------- prologue ----------------
        emit_p1(0)
        emit_p1(1)

        hp_prev = s_hp.tile([128, H], F32R, tag="hp")
        nc.vector.tensor_copy(hp_prev[64:128, :], h0_r[:].bitcast(F32))
        hp_cur = None

        yt_chunks = {}

        # ---------------- scan + interleaved phases ----------------
        for t in range(T):
            rc = t // TCH
            if t % TCH == 0:
                if rc + 2 < NCHUNK:
                    emit_p1(rc + 2)
                if rc >= 1:
                    emit_p3(rc - 1, yt_chunks.pop(rc - 1))
                yt_chunks[rc] = ytp.tile([128, 4 * RCH], F32R, tag="ytc", name="ytc")

            parity = t % 2
            pr = 64 * (1 - parity)          # partition offset of cur_h in its pair tile
            if parity == 0:
                hp_cur = s_hp.tile([128, H], F32R, tag="hp")
                cur_h = hp_prev[64:128, :]
            else:
                cur_h = hp_cur[0:64, :]

            xi_sb = s_in.tile([BL, 3 * H], F32, tag="xi")
            nc.sync.dma_start(xi_sb[:], d_xi[t])
            dg_sb = s_in.tile([128, 65], F32R, tag="dg")
            nc.sync.dma_start(dg_sb[:], d_dgnd[t])

            # hmT [128, 256] f32r: k-tile k at cols 64k = (cur_h * nd).T rows 128k..
            psT = s_psT.tile([128, 256], F32, tag="psT")
            for k in range(4):
                nc.tensor.matmul(psT[:, 64 * k:64 * k + 64],
                                 lhsT=cur_h[:, 128 * k:128 * k + 128],
                                 rhs=dg_sb[pr:pr + 64, 0:64],
                                 start=(k == 0), stop=(k == 3),
                                 skip_group_check=True)
            hmT_r = s_sb2.tile([128, 256], F32R, tag="hmT")
            nc.scalar.copy(hmT_r[:], psT[:])

            # gate matmuls: z, r, n (chain needs z early for zc, n last)
            ps_rz = s_ps.tile([64, 1024], F32, tag="psrz")   # [:,0:512]=r, [:,512:1024]=z
            ps_n = s_ps.tile([64, 512], F32, tag="psn")
            for k in range(4):
                nc.tensor.matmul(ps_rz[:, 512:1024], lhsT=hmT_r[:, 64 * k:64 * k + 64],
                                 rhs=wh_r[:, k * 1536 + 512:k * 1536 + 1024],
                                 start=(k == 0), stop=False, skip_group_check=True)
            for k in range(4):
                nc.tensor.matmul(ps_rz[:, 0:512], lhsT=hmT_r[:, 64 * k:64 * k + 64],
                                 rhs=wh_r[:, k * 1536:k * 1536 + 512],
                                 start=(k == 0), stop=(k == 3), skip_group_check=True)
            for k in range(4):
                nc.tensor.matmul(ps_n[:], lhsT=hmT_r[:, 64 * k:64 * k + 64],
                                 rhs=wh_r[:, k * 1536 + 1024:k * 1536 + 1536],
                                 start=(k == 0), stop=False, skip_group_check=True)
            nc.tensor.matmul(ps_n[:], lhsT=ones_r[:, 0:64], rhs=bhn_r[:],
                             start=False, stop=True, skip_group_check=True)

            # elementwise: h_new = 2*(zc*s) + (z*nd*h - zc) ; s = sigmoid(2*npre)
            z_act = s_sb.tile([64, 512], F32, tag="z_act")
            nc.vector.tensor_tensor(z_act[:], ps_rz[:, 512:1024], xi_sb[:, 512:1024], ALU.add)
            zc_sig = s_sb.tile([64, 512], F32, tag="zc_sig")
            nc.scalar.activation(zc_sig[:], z_act[:], AF.Sigmoid, scale=-1.0)
            z_sig = s_sb.tile([64, 512], F32, tag="z_sig")
            nc.scalar.activation(z_sig[:], z_act[:], AF.Sigmoid)
            hm_t = s_sb.tile([64, 512], F32, tag="hm_t")
            nc.vector.tensor_scalar_mul(hm_t[:], cur_h[:].bitcast(F32),
                                        dg_sb[pr:pr + 64, 64:65].bitcast(F32))
            zh = s_sb.tile([64, 512], F32, tag="zh")
            nc.vector.tensor_tensor(zh[:], z_sig[:], hm_t[:], ALU.mult)
            w_t = s_sb.tile([64, 512], F32, tag="w_t")
            nc.vector.tensor_tensor(w_t[:], zh[:], zc_sig[:], ALU.subtract)
            r_act = s_sb.tile([64, 512], F32, tag="r_act")
            nc.vector.tensor_tensor(r_act[:], ps_rz[:, 0:512], xi_sb[:, 0:512], ALU.add)
            r_sig = s_sb.tile([64, 512], F32, tag="r_sig")
            nc.scalar.activation(r_sig[:], r_act[:], AF.Sigmoid)
            t2 = s_sb.tile([64, 512], F32, tag="t2")
            nc.vector.tensor_tensor(t2[:], r_sig[:], ps_n[:], ALU.mult)
            n_pre = s_sb.tile([64, 512], F32, tag="n_pre")
            nc.vector.tensor_tensor(n_pre[:], t2[:], xi_sb[:, 1024:1536], ALU.add)
            s_sig = s_sb.tile([64, 512], F32, tag="s_sig")
            nc.scalar.activation(s_sig[:], n_pre[:], AF.Sigmoid, scale=2.0)
            u_t = s_sb.tile([64, 512], F32, tag="u_t")
            nc.vector.tensor_tensor(u_t[:], zc_sig[:], s_sig[:], ALU.mult)
            pr2 = 64 * parity
            nc.vector.scalar_tensor_tensor(hp_cur[pr2:pr2 + 64, :], u_t[:], 2.0, w_t[:],
                                           ALU.mult, ALU.add)

            if parity == 1:
                # yT for the pair (t-1, t) -> yt_chunk cols [128*p4 : +128] of each k block
                p4 = (t % TCH) // 2
                psY = s_psY.tile([128, 512], F32, tag="psY")
                for fh in range(4):
                    nc.tensor.matmul(psY[:, 128 * fh:128 * fh + 128],
                                     lhsT=hp_cur[:, 128 * fh:128 * fh + 128],
                                     rhs=eye_r[:], start=(fh == 0), stop=(fh == 3),
                                     skip_group_check=True)
                dst = yt_chunks[rc][:].rearrange("p (f r) -> p f r", f=4)[:, :, 128 * p4:128 * p4 + 128]
                nc.scalar.copy(dst, psY[:].rearrange("p (f r) -> p f r", f=4))
                hp_prev = hp_cur

        nc.sync.dma_start(o_hid[:], hp_prev[64:128, :].bitcast(F32))
        emit_p3(NCHUNK - 1, yt_chunks.pop(NCHUNK - 1))

        nc.sync.dma_start(o_demb[:], acc_emb[:])
        nc.sync.dma_start(o_dact[:], acc_act[:])
        nc.sync.dma_start(o_dcr[:], acc_cr[:])
        nc.sync.dma_start(o_absy[:], acc_absy[:])
        es.close()

    nc.compile()
    return nc


def _get_program():
    global _PROGRAM
    if _PROGRAM is None:
        _PROGRAM = _build_program()
    return _PROGRAM


def make_in_maps(hidden, obs, dones, W_emb, b_emb, Wi, bi, Wh, bhn,
                 W_act, b_act, W_act_out, b_act_out,
                 W_cr, b_cr, W_cr_out, b_cr_out):
    f32 = np.float32
    hidden = np.asarray(hidden, f32)
    obs = np.asarray(obs, f32)
    nd = (1.0 - np.asarray(dones, f32)).astype(f32)          # [T, B]
    bibc = np.broadcast_to(np.asarray(bi, f32)[None, :], (128, 3 * H)).copy()
    eye128 = np.eye(128, dtype=f32)
    shared = dict(
        wemb=np.ascontiguousarray(W_emb, f32), wi=np.ascontiguousarray(Wi, f32),
        wh=np.ascontiguousarray(Wh, f32), wact=np.ascontiguousarray(W_act, f32),
        wcr=np.ascontiguousarray(W_cr, f32), wao=np.ascontiguousarray(W_act_out, f32),
        wco=np.ascontiguousarray(W_cr_out, f32),
        bemb=np.ascontiguousarray(np.asarray(b_emb, f32).reshape(4, 128).T),
        bibc=bibc,
        bhn=np.ascontiguousarray(np.asarray(bhn, f32)[None, :]),
        bact=np.ascontiguousarray(np.asarray(b_act, f32).reshape(4, 128).T),
        bcr=np.ascontiguousarray(np.asarray(b_cr, f32).reshape(4, 128).T),
        bao=np.ascontiguousarray(np.asarray(b_act_out, f32).reshape(A, 1)),
        bco=np.ascontiguousarray(np.asarray(b_cr_out, f32).reshape(1, 1)),
        eye128=eye128,
    )
    ii = np.arange(64)
    in_maps = []
    for c in range(NC):
        sl = slice(c * BL, (c + 1) * BL)
        obs_c = obs[:, sl, :]
        obsT = np.ascontiguousarray(obs_c.transpose(2, 0, 1).reshape(OBS, T * BL))
        nd_c = nd[:, sl]                                       # [T, 64]
        dgnd = np.zeros((T, 128, 65), f32)
        dgnd[:, ii, ii] = nd_c
        dgnd[:, 64 + ii, ii] = nd_c
        dgnd[:, :64, 64] = nd_c
        dgnd[:, 64:, 64] = nd_c
        in_maps.append(dict(shared, obsT=obsT, h0f=np.ascontiguousarray(hidden[sl]),
                            dgnd=dgnd))
    return in_maps


def postprocess(results, log_std):
    f32 = np.float32
    hidden = np.concatenate([r["hid"] for r in results], axis=0)             # [B, H]
    actor_mean = np.concatenate(
        [np.ascontiguousarray(r["am"].transpose(1, 2, 0)) for r in results], axis=1)
    value = np.concatenate([r["val"] for r in results], axis=1)              # [T, B]
    sigma = np.exp(np.asarray(log_std, f32)).astype(f32)

    def unstat(key):  # -> [B, 512] ; stat[p, 64k+b] = val(b of this core, 128k+p)
        full = np.empty((B, 512), f32)
        for c, r in enumerate(results):
            s = r[key].reshape(128, 4, 64)                    # [p, k, b]
            full[c * BL:(c + 1) * BL] = s.transpose(2, 1, 0).reshape(BL, 512)
        return full

    d_emb = unstat("demb")
    d_act = unstat("dact")
    d_cr = unstat("dcr")
    d_absy = unstat("absy")

    def dorm(d_sum, lead, layer_dim):
        d = (d_sum / f32(lead)).astype(f32)
        s = (d.sum(dtype=f32) / f32(layer_dim) + f32(1e-8)).astype(f32)
        mask = (d / s) <= f32(0.0)
        return f32(mask.sum() / layer_dim * 100)

    ad1 = dorm(d_act, T, FC)
    ed1 = dorm(d_emb, T, FC)
    dh = np.abs(hidden).sum(axis=0, dtype=f32)
    sh = (dh / f32(B))
    hd1 = f32(((sh / (sh.sum(dtype=f32) / f32(H) + f32(1e-8))) <= 0).sum() / H * 100)
    ed2 = dorm(d_absy, T, H)
    cd1 = dorm(d_cr, T, 256)
    return (hidden, actor_mean, sigma, value, ad1, ed1, hd1, ed2, cd1)


def kernel(hidden, obs, dones, W_emb, b_emb, Wi, bi, Wh, bhn,
           W_act, b_act, W_act_out, b_act_out, log_std,
           W_cr, b_cr, W_cr_out, b_cr_out):
    from concourse.bass_utils import run_bass_kernel_spmd
    nc = _get_program()
    in_maps = make_in_maps(hidden, obs, dones, W_emb, b_emb, Wi, bi, Wh, bhn,
                           W_act, b_act, W_act_out, b_act_out,
                           W_cr, b_cr, W_cr_out, b_cr_out)
    res = run_bass_kernel_spmd(nc, in_maps, core_ids=list(range(NC)))
    return postprocess(res.results, log_std)
